# revision 8
# baseline (speedup 1.0000x reference)
"""Trainium2 Bass kernel for a 2-layer GCN encoder (DMoN encoder).

Math (reference):
    row/col = edge_index with self-loops appended; deg = in-degree over row.
    norm_e = deg[row_e]^-1/2 * deg[col_e]^-1/2
    gcn(h, W):  segsum_row(norm * (h W)[col])  (+bias, absorbed by BN)
    h1 = relu(bn(gcn(x,  W1)));  h2 = relu(bn(gcn(h1, W2)))
    S  = softmax(relu(h2 AW1 + Ab1) AW2 + Ab2)
    return h2, S

Device mapping (8 NeuronCores, node-partitioned):
    * Each core owns a contiguous range of N/8 nodes (all edges targeting them).
    * Aggregation uses the identity  (A @ h) W == A @ (h W):  gather RAW rows
      h[col] with the Ant dma_gather (int16 indices -> node space split into
      4 ranges of N/4), then segment-sum via one-hot matmul:
          S1h[e, d] = norm_e * (rowlocal_e == d)   (one DVE tensor_scalar op)
          agg[f, d] += msg[e, f]^T @ S1h[e, d]     (PE, accumulated in PSUM)
      F-major agg -> transform via lhsT=W matmul, BN stats via ACT accum_out,
      global BN moments via AllReduce, apply scale/bias+relu in one ACT op.
    * h1 is AllGathered so layer 2 can gather remote rows; h2/S are written
      per-core and concatenated on the host.
"""

import math

import numpy as np

import concourse.bass as bass
import concourse.mybir as mybir
import concourse.tile as tile
from concourse import bacc
from concourse.bass_utils import run_bass_kernel_spmd
from concourse.masks import make_identity

F32 = mybir.dt.float32
I16 = mybir.dt.int16

# problem constants (hardcoded per contract)
N_FULL, E_FULL, D, HID, K = 100000, 1600000, 128, 128, 16
NCORES = 8
BN_EPS = 1e-5

# tunables
W_BLK = 96          # dest-window (psum free dim) per aggregation block
NRANGES = 4         # source ranges (int16 gather index limit)
G_IDX = 1024        # gather indices per dma_gather instruction (ring limit <1536)
MSG_BUFS = 3        # gather-tile double buffering per range
SKIP_COLLECTIVES = False  # debug: replace collectives with local copies
P = 128


def _preprocess(edge_index, n, ncores, w_blk, nranges):
    """Host-side sharding: per-core, per-source-range edge streams sorted by
    destination block, each (block, range) cell padded to a multiple of 128
    edges, cell slot-counts unified across cores (SPMD needs one program).

    Returns (meta, per_core) where
      meta = dict(k_max [nblocks, nranges] slots per cell, L [nranges] stream
                  lengths, n_own, nblocks, rng_size)
      per_core[c] = dict(idx16[r], rowlocal[r], norm[r]) arrays.
    """
    row = np.asarray(edge_index[0], dtype=np.int64)
    col = np.asarray(edge_index[1], dtype=np.int64)
    e = row.shape[0]
    deg = (np.bincount(row, minlength=n) + 1).astype(np.float32)
    dinv = (1.0 / np.sqrt(deg)).astype(np.float32)

    loops = np.arange(n, dtype=np.int64)
    rows = np.concatenate([row, loops])
    cols = np.concatenate([col, loops])
    norms = (dinv[rows] * dinv[cols]).astype(np.float32)

    n_own = n // ncores
    nblocks = math.ceil(n_own / w_blk)
    rng_size = n // nranges

    core_of = rows // n_own
    block_of = (rows % n_own) // w_blk
    range_of = cols // rng_size

    counts = np.zeros((ncores, nblocks, nranges), np.int64)
    np.add.at(counts, (core_of, block_of, range_of), 1)
    k = -(-counts // P)                     # slots (128-edge sub-chunks) per cell
    k_max = k.max(axis=0)                   # [nblocks, nranges]
    # stream offsets (in edges) per range: cells in block order
    offs = np.zeros((nblocks, nranges), np.int64)
    for r in range(nranges):
        offs[:, r] = P * np.concatenate([[0], np.cumsum(k_max[:-1, r])])
    L = P * k_max.sum(axis=0)               # [nranges]

    per_core = []
    for c in range(ncores):
        m = core_of == c
        cb = block_of[m]
        cr = range_of[m]
        cc = cols[m]
        cn = norms[m]
        cl = (rows[m] % n_own) % w_blk
        order = np.lexsort((cr, cb))
        cb, cr, cc, cn, cl = cb[order], cr[order], cc[order], cn[order], cl[order]
        # rank within cell
        cell = cb * nranges + cr
        change = np.r_[True, cell[1:] != cell[:-1]]
        seg_start = np.flatnonzero(change)
        seg_id = np.cumsum(change) - 1
        rank = np.arange(cell.shape[0]) - seg_start[seg_id]
        pos = offs[cb, cr] + rank

        entry = {"idx16": [], "rowlocal": [], "norm": []}
        for r in range(nranges):
            sel = cr == r
            lr = int(L[r])
            fi = np.zeros(lr, np.int16)
            fn = np.zeros(lr, np.float32)
            fl = np.zeros(lr, np.float32)
            p_r = pos[sel]
            fi[p_r] = (cc[sel] - r * rng_size).astype(np.int16)
            fn[p_r] = cn[sel]
            fl[p_r] = cl[sel].astype(np.float32)
            # wrap layouts: idx i -> [i%16, i//16]; edge i -> [i%128, i//128]
            entry["idx16"].append(np.ascontiguousarray(
                np.tile(fi.reshape(lr // 16, 16).T, (8, 1))))
            entry["rowlocal"].append(np.ascontiguousarray(
                fl.reshape(lr // P, P).T))
            entry["norm"].append(np.ascontiguousarray(
                fn.reshape(lr // P, P).T))
        per_core.append(entry)

    meta = dict(k_max=k_max, L=L, n_own=n_own, nblocks=nblocks,
                rng_size=rng_size, e=e)
    return meta, per_core


def _build_program(meta, n, ncores, w_blk, nranges, g_idx):
    n_own = meta["n_own"]
    nblocks = meta["nblocks"]
    rng_size = meta["rng_size"]
    k_max = meta["k_max"]
    L = meta["L"]

    nc = bacc.Bacc("TRN2", target_bir_lowering=False, debug=False,
                   num_devices=ncores)

    x_d = nc.dram_tensor("x", [n, D], F32, kind="ExternalInput")
    w1_d = nc.dram_tensor("W1", [D, HID], F32, kind="ExternalInput")
    w2_d = nc.dram_tensor("W2", [HID, HID], F32, kind="ExternalInput")
    aw1_d = nc.dram_tensor("AW1", [HID, HID], F32, kind="ExternalInput")
    aw2_d = nc.dram_tensor("AW2", [HID, K], F32, kind="ExternalInput")
    vec_d = {}
    for name, dim in [("g1", HID), ("be1", HID), ("g2", HID), ("be2", HID),
                      ("Ab1", HID), ("Ab2", K)]:
        vec_d[name] = nc.dram_tensor(name, [dim], F32, kind="ExternalInput")
    idx_d, rl_d, nm_d = [], [], []
    for r in range(nranges):
        lr = int(L[r])
        idx_d.append(nc.dram_tensor(f"idx{r}", [P, lr // 16], I16,
                                    kind="ExternalInput"))
        rl_d.append(nc.dram_tensor(f"rl{r}", [P, lr // P], F32,
                                   kind="ExternalInput"))
        nm_d.append(nc.dram_tensor(f"nm{r}", [P, lr // P], F32,
                                   kind="ExternalInput"))
    h2_d = nc.dram_tensor("h2_own", [n_own, D], F32, kind="ExternalOutput")
    s_d = nc.dram_tensor("s_own", [n_own, K], F32, kind="ExternalOutput")

    slots_per_g = g_idx // P

    with tile.TileContext(nc) as tc:
        with (
            tc.tile_pool(name="const", bufs=1) as constp,
            tc.tile_pool(name="stream", bufs=1) as streamp,
            tc.tile_pool(name="msg", bufs=MSG_BUFS) as msgp,
            tc.tile_pool(name="idxp", bufs=3) as idxp,
            tc.tile_pool(name="s1h", bufs=4) as s1hp,
            tc.tile_pool(name="work", bufs=3) as workp,
            tc.tile_pool(name="hconv", bufs=1) as hconvp,
            tc.tile_pool(name="stat", bufs=1) as statp,
            tc.tile_pool(name="psA", bufs=2, space="PSUM") as psA,
            tc.tile_pool(name="psB", bufs=2, space="PSUM") as psB,
            tc.tile_pool(name="psC", bufs=2, space="PSUM") as psC,
            tc.tile_pool(name="psD", bufs=2, space="PSUM") as psD,
            tc.tile_pool(name="dram", bufs=1, space="DRAM") as dramp,
        ):
            # ---- constants ----
            iota_t = constp.tile([P, w_blk], F32)
            nc.gpsimd.iota(iota_t[:], pattern=[[1, w_blk]], base=0,
                           channel_multiplier=0,
                           allow_small_or_imprecise_dtypes=True)
            ident = constp.tile([P, P], F32)
            make_identity(nc, ident[:])
            w1_sb = constp.tile([D, HID], F32)
            nc.sync.dma_start(out=w1_sb[:], in_=w1_d[:, :])
            w2_sb = constp.tile([HID, HID], F32)
            nc.sync.dma_start(out=w2_sb[:], in_=w2_d[:, :])
            aw1_sb = constp.tile([HID, HID], F32)
            nc.sync.dma_start(out=aw1_sb[:], in_=aw1_d[:, :])
            aw2_sb = constp.tile([HID, K], F32)
            nc.sync.dma_start(out=aw2_sb[:], in_=aw2_d[:, :])
            vec_sb = {}
            for name, dim in [("g1", HID), ("be1", HID), ("g2", HID),
                              ("be2", HID), ("Ab1", HID), ("Ab2", K)]:
                t = constp.tile([dim, 1], F32, tag=f"vec_{name}")
                nc.sync.dma_start(
                    out=t[:], in_=vec_d[name][:].rearrange("(p o) -> p o", o=1))
                vec_sb[name] = t

            # ---- resident index/meta streams ----
            rl_sb, nm_sb = [], []
            for r in range(nranges):
                lr = int(L[r])
                t = streamp.tile([P, lr // P], F32, tag=f"rl{r}")
                nc.sync.dma_start(out=t[:], in_=rl_d[r][:, :])
                rl_sb.append(t)
                t = streamp.tile([P, lr // P], F32, tag=f"nm{r}")
                nc.sync.dma_start(out=t[:], in_=nm_d[r][:, :])
                nm_sb.append(t)

            # ---- internal DRAM ----
            h1_own_b = dramp.tile([n_own, D], F32)
            h1_full = dramp.tile([n, D], F32)
            stat_in_b = dramp.tile([P, 2], F32)
            stat_out_b = dramp.tile([P, 2], F32)

            def layer(li, table_ap, wt_sb, g_sb, be_sb, head):
                """One GCN layer. table_ap: DRAM [n, D] gather source.
                Returns nothing; writes h_out rows to (h1_own_b | h2_d) and,
                if head, S rows to s_d."""
                hconv = hconvp.tile([P, n_own], F32, tag="hconv")
                stat_sum = statp.tile([P, nblocks], F32, tag="ssum")
                stat_sq = statp.tile([P, nblocks], F32, tag="ssq")

                # issue all gathers for this layer
                msg_tiles = [[] for _ in range(nranges)]
                for r in range(nranges):
                    lr = int(L[r])
                    n_g = math.ceil(lr / g_idx)
                    for g in range(n_g):
                        i0 = g * g_idx
                        ni = min(g_idx, lr - i0)
                        it = idxp.tile([P, g_idx // 16], I16, tag=f"it{r}")
                        nc.sync.dma_start(
                            out=it[:, :ni // 16],
                            in_=idx_d[r][:, i0 // 16:(i0 + ni) // 16])
                        mt = msgp.tile([P, slots_per_g, D], F32, tag=f"msg{r}")
                        nc.gpsimd.dma_gather(
                            out_ap=mt[:, :ni // P, :],
                            in_ap=table_ap[r * rng_size:(r + 1) * rng_size, :],
                            idxs_ap=it[:, :ni // 16],
                            num_idxs=ni,
                            num_idxs_reg=ni,
                            elem_size=D,
                        )
                        msg_tiles[r].append(mt)

                # aggregation + transform per block
                cur = [0] * nranges
                for b in range(nblocks):
                    wb = min(w_blk, n_own - b * w_blk)
                    total = int(k_max[b].sum())
                    agg_ps = psA.tile([P, w_blk], F32, tag="agg")
                    mm = 0
                    for r in range(nranges):
                        for _ in range(int(k_max[b, r])):
                            s = cur[r]
                            cur[r] += 1
                            g, j = divmod(s, slots_per_g)
                            s1h = s1hp.tile([P, w_blk], F32, tag="s1h")
                            nc.vector.tensor_scalar(
                                out=s1h[:, :wb], in0=iota_t[:, :wb],
                                scalar1=rl_sb[r][:, s:s + 1],
                                scalar2=nm_sb[r][:, s:s + 1],
                                op0=mybir.AluOpType.is_equal,
                                op1=mybir.AluOpType.mult,
                            )
                            nc.tensor.matmul(
                                agg_ps[:, :wb],
                                msg_tiles[r][g][:, j, :],
                                s1h[:, :wb],
                                start=(mm == 0), stop=(mm == total - 1),
                            )
                            mm += 1
                    agg_sb = workp.tile([P, w_blk], F32, tag="aggsb")
                    nc.scalar.copy(agg_sb[:, :wb], agg_ps[:, :wb])
                    tr_ps = psB.tile([P, w_blk], F32, tag="tr")
                    nc.tensor.matmul(tr_ps[:, :wb], wt_sb[:], agg_sb[:, :wb],
                                     start=True, stop=True)
                    # psum -> hconv slice, with row-sum / row-sumsq stats
                    nc.scalar.activation(
                        out=hconv[:, b * w_blk:b * w_blk + wb],
                        in_=tr_ps[:, :wb],
                        func=mybir.ActivationFunctionType.Copy,
                        accum_out=stat_sum[:, b:b + 1])
                    sq_t = workp.tile([P, w_blk], F32, tag="sq")
                    nc.scalar.activation(
                        out=sq_t[:, :wb], in_=tr_ps[:, :wb],
                        func=mybir.ActivationFunctionType.Square,
                        accum_out=stat_sq[:, b:b + 1])

                # global BN moments
                stat2 = statp.tile([P, 2], F32, tag="stat2")
                nc.vector.tensor_reduce(stat2[:, 0:1], stat_sum[:],
                                        axis=mybir.AxisListType.X,
                                        op=mybir.AluOpType.add)
                nc.vector.tensor_reduce(stat2[:, 1:2], stat_sq[:],
                                        axis=mybir.AxisListType.X,
                                        op=mybir.AluOpType.add)
                nc.sync.dma_start(out=stat_in_b[:], in_=stat2[:])
                if SKIP_COLLECTIVES:
                    nc.sync.dma_start(out=stat_out_b[:], in_=stat_in_b[:])
                else:
                    nc.gpsimd.collective_compute(
                        "AllReduce", mybir.AluOpType.add,
                        replica_groups=[list(range(ncores))],
                        ins=[stat_in_b.opt()], outs=[stat_out_b.opt()],
                    )
                statg = statp.tile([P, 2], F32, tag="statg")
                nc.sync.dma_start(out=statg[:], in_=stat_out_b[:])
                mu = statp.tile([P, 1], F32, tag="mu")
                nc.vector.tensor_scalar_mul(mu[:], statg[:, 0:1], 1.0 / n)
                var = statp.tile([P, 1], F32, tag="var")
                nc.vector.tensor_scalar_mul(var[:], statg[:, 1:2], 1.0 / n)
                musq = statp.tile([P, 1], F32, tag="musq")
                nc.vector.tensor_tensor(out=musq[:], in0=mu[:], in1=mu[:],
                                        op=mybir.AluOpType.mult)
                nc.vector.tensor_tensor(out=var[:], in0=var[:], in1=musq[:],
                                        op=mybir.AluOpType.subtract)
                nc.vector.tensor_scalar_add(var[:], var[:], float(BN_EPS))
                std = statp.tile([P, 1], F32, tag="std")
                nc.scalar.activation(out=std[:], in_=var[:],
                                     func=mybir.ActivationFunctionType.Sqrt)
                rstd = statp.tile([P, 1], F32, tag="rstd")
                nc.vector.reciprocal(rstd[:], std[:])
                sc = statp.tile([P, 1], F32, tag="sc")
                nc.vector.tensor_tensor(out=sc[:], in0=rstd[:], in1=g_sb[:],
                                        op=mybir.AluOpType.mult)
                tsh = statp.tile([P, 1], F32, tag="tsh")
                nc.vector.tensor_tensor(out=tsh[:], in0=mu[:], in1=sc[:],
                                        op=mybir.AluOpType.mult)
                nc.vector.tensor_tensor(out=tsh[:], in0=be_sb[:], in1=tsh[:],
                                        op=mybir.AluOpType.subtract)

                # pass B: bn+relu in place (512-wide)
                c0 = 0
                while c0 < n_own:
                    cw = min(512, n_own - c0)
                    nc.scalar.activation(
                        out=hconv[:, c0:c0 + cw], in_=hconv[:, c0:c0 + cw],
                        func=mybir.ActivationFunctionType.Relu,
                        bias=tsh[:], scale=sc[:])
                    c0 += cw

                # transpose + write out rows (+ head)
                h_out_rows = h2_d if li == 2 else h1_own_b
                c0 = 0
                while c0 < n_own:
                    cw = min(P, n_own - c0)
                    hT_ps = psC.tile([P, P], F32, tag="hT")
                    nc.tensor.transpose(hT_ps[:cw, :], hconv[:, c0:c0 + cw],
                                        ident[:])
                    hT_sb = workp.tile([P, P], F32, tag="hTsb")
                    nc.scalar.copy(hT_sb[:cw, :], hT_ps[:cw, :])
                    nc.sync.dma_start(out=h_out_rows[c0:c0 + cw, :],
                                      in_=hT_sb[:cw, :])
                    if head:
                        z1_ps = psD.tile([P, P], F32, tag="z1")
                        nc.tensor.matmul(z1_ps[:, :cw], aw1_sb[:],
                                         hconv[:, c0:c0 + cw],
                                         start=True, stop=True)
                        z1_sb = workp.tile([P, P], F32, tag="z1sb")
                        nc.scalar.activation(
                            out=z1_sb[:, :cw], in_=z1_ps[:, :cw],
                            func=mybir.ActivationFunctionType.Relu,
                            bias=vec_sb["Ab1"][:])
                        lg_ps = psD.tile([K, P], F32, tag="z1")
                        nc.tensor.matmul(lg_ps[:, :cw], aw2_sb[:],
                                         z1_sb[:, :cw], start=True, stop=True)
                        lg_sb = workp.tile([K, P], F32, tag="lgsb")
                        nc.scalar.activation(
                            out=lg_sb[:, :cw], in_=lg_ps[:, :cw],
                            func=mybir.ActivationFunctionType.Identity,
                            bias=vec_sb["Ab2"][:])
                        lgT_ps = psC.tile([P, K], F32, tag="hT")
                        nc.tensor.transpose(lgT_ps[:cw, :], lg_sb[:, :cw],
                                            ident[:K, :K])
                        exp_sb = workp.tile([P, K], F32, tag="expsb")
                        sume = workp.tile([P, 1], F32, tag="sume")
                        nc.scalar.activation(
                            out=exp_sb[:cw, :], in_=lgT_ps[:cw, :],
                            func=mybir.ActivationFunctionType.Exp,
                            accum_out=sume[:cw, :])
                        rec = workp.tile([P, 1], F32, tag="rec")
                        nc.vector.reciprocal(rec[:cw, :], sume[:cw, :])
                        smx = workp.tile([P, K], F32, tag="smx")
                        nc.vector.tensor_scalar_mul(smx[:cw, :], exp_sb[:cw, :],
                                                    rec[:cw, :])
                        nc.sync.dma_start(out=s_d[c0:c0 + cw, :],
                                          in_=smx[:cw, :])
                    c0 += cw

            layer(1, x_d[:, :], w1_sb, vec_sb["g1"], vec_sb["be1"], head=False)
            if SKIP_COLLECTIVES:
                nc.sync.dma_start(out=h1_full[0:n_own, :], in_=h1_own_b[:, :])
            else:
                nc.gpsimd.collective_compute(
                    "AllGather", mybir.AluOpType.bypass,
                    replica_groups=[list(range(ncores))],
                    ins=[h1_own_b.opt()], outs=[h1_full.opt()],
                )
            layer(2, h1_full[:, :], w2_sb, vec_sb["g2"], vec_sb["be2"],
                  head=True)

    nc.compile()
    return nc


def _prepare(x, edge_index, W1, W2, AW1, AW2, g1, be1, g2, be2, Ab1, Ab2,
             n, ncores, verbose=False):
    import time as _time
    _t = _time.time()
    meta, per_core = _preprocess(edge_index, n, ncores, W_BLK, NRANGES)
    if verbose:
        print(f"[pipeline] preprocess {_time.time()-_t:.1f}s", flush=True)
    _t = _time.time()
    nc = _build_program(meta, n, ncores, W_BLK, NRANGES, G_IDX)
    if verbose:
        print(f"[pipeline] build+schedule+bacc {_time.time()-_t:.1f}s", flush=True)

    base = {
        "x": np.ascontiguousarray(x, dtype=np.float32),
        "W1": np.ascontiguousarray(W1, dtype=np.float32),
        "W2": np.ascontiguousarray(W2, dtype=np.float32),
        "AW1": np.ascontiguousarray(AW1, dtype=np.float32),
        "AW2": np.ascontiguousarray(AW2, dtype=np.float32),
        "g1": np.ascontiguousarray(g1, dtype=np.float32),
        "be1": np.ascontiguousarray(be1, dtype=np.float32),
        "g2": np.ascontiguousarray(g2, dtype=np.float32),
        "be2": np.ascontiguousarray(be2, dtype=np.float32),
        "Ab1": np.ascontiguousarray(Ab1, dtype=np.float32),
        "Ab2": np.ascontiguousarray(Ab2, dtype=np.float32),
    }
    in_maps = []
    for c in range(ncores):
        m = dict(base)
        for r in range(NRANGES):
            m[f"idx{r}"] = per_core[c]["idx16"][r]
            m[f"rl{r}"] = per_core[c]["rowlocal"][r]
            m[f"nm{r}"] = per_core[c]["norm"][r]
        in_maps.append(m)
    return nc, in_maps


def _execute(nc, in_maps, ncores, trace=False, verbose=False):
    import time as _time
    _t = _time.time()
    res = run_bass_kernel_spmd(nc, in_maps, core_ids=list(range(ncores)),
                               trace=trace)
    if verbose:
        print(f"[pipeline] compile+run {_time.time()-_t:.1f}s", flush=True)
    h2 = np.concatenate([res.results[c]["h2_own"] for c in range(ncores)], 0)
    s = np.concatenate([res.results[c]["s_own"] for c in range(ncores)], 0)
    return (h2, s), res


def _run_pipeline(x, edge_index, W1, W2, AW1, AW2, g1, be1, g2, be2, Ab1, Ab2,
                  n, ncores, trace=False, verbose=False):
    nc, in_maps = _prepare(x, edge_index, W1, W2, AW1, AW2, g1, be1, g2, be2,
                           Ab1, Ab2, n, ncores, verbose=verbose)
    return _execute(nc, in_maps, ncores, trace=trace, verbose=verbose)


def kernel(x, edge_index, W1, b1, g1, be1, W2, b2, g2, be2, AW1, Ab1, AW2, Ab2):
    # b1/b2 are mathematically absorbed by the following batchnorm (the mean
    # shift cancels) so they are not shipped to the device.
    (h2, s), _ = _run_pipeline(
        np.asarray(x), np.asarray(edge_index), np.asarray(W1), np.asarray(W2),
        np.asarray(AW1), np.asarray(AW2), np.asarray(g1), np.asarray(be1),
        np.asarray(g2), np.asarray(be2), np.asarray(Ab1), np.asarray(Ab2),
        n=N_FULL, ncores=NCORES)
    return h2, s
